# revision 41
# baseline (speedup 1.0000x reference)
"""Trainium2 Bass kernel: EnergyConditionedFieldAttention.

Sharding: data-parallel over batch B=64 across 8 NeuronCores (8 batches
per core). MLP weights and the shared query path q = mlp3(e_feat) are
replicated on every core; each core returns out[8, 500, 256] and the
host concatenates.

Structural choices over the f32r baseline (259 us):

1. Token compaction. The mask is ~Bernoulli(0.5) per token and masked
   tokens are dead in every term (scores -1e9 AND post-softmax mask),
   so the host gathers unmasked field rows per batch into CAP=336
   columns (mean count 256, sigma 11.3 -> 7 sigma headroom; if a mask
   ever exceeds CAP the kernel transparently rebuilds at 512). A 0/1
   pad-mask column replaces the token mask on-device. The host also
   pre-transposes the gathered field to [2, 128, CAP] (layout-only
   work, like the per-core batch slicing), killing the per-batch PE
   transposes.

2. fp8 (e4m3) DoubleRow matmuls on the score path. DoubleRow contracts
   two 128-row k-tiles per pass at ~4x f32r row rate. Scaled scores
   satisfy |s| <= ~0.03, so relative error eps in q/k perturbs
   attention weights by only ~eps*|s|: the q-MLP (L2/L3), k-MLP and
   scores run in fp8 adding just ~2e-5 output error (measured against
   an exact numpy emulation). Weights are prescaled x16 on-device
   before the fp8 cast (uniform-init weights sit below the e4m3
   normal range; x16 lifts them to [0.25, 1] clear of any FTZ), and
   the inverse 1/16 rides the next activation's scale port. The value
   path (v-MLP, U, out-MLP) stays f32r: attention averaging does not
   suppress v-side noise (fp8 there costs ~3% output error).

3. One-batch software skew. Stage A(b+1) = k/v MLP (ACT-heavy silu
   chains) is interleaved phase-by-phase into stage B(b) = scores/
   attention/out-MLP (PE/DVE-heavy). Separate PSUM pools per stream
   (ps_mm vs ps_s/ps_u) keep a silu drain from ever stalling the PE:
   it just runs the other stream's matmuls.

4. All f32r weights/field are declared float32r in DRAM (same 32 bits,
   numpy float32 on host) so they ride the two fast HWDGE rings (sync
   + scalar) with no SWDGE cast; staging for the fp8 casts interleaves
   with batch-0's MLP so the PE starts ~3 us into the kernel.

Per-core plan (PSUM accumulation fp32 throughout):
  qTs8 = mlp3_fp8(e_featT)       [128, 2, NEP]  once; raw q, fp8
  qwT8 = W3_k^T qTs8             [128, 4, NEP]  once; k L3 folded into
         scores: s = kh2 . qwT. The dropped q.b3 term is a per-energy
         constant that cancels in softmax; the poly-exp argument
         actually shrinks (|x| <= 0.021). Kills the per-batch k L3
         matmuls AND the kT8 fp8 cast on the iteration-tail chain.
  per local batch b:
    kh*  = fp8 silu chain        [128, 4, CAP]  DoubleRow
    sT   = kh2_8^T (x) qwT8      [tok, NEP]     DoubleRow, raw scores
    y    = ((c1*S*s+c2)^2+c3)*padmask           ACT Square + DVE
    v    = f32r mlp3             v_aug [128, 3, 272] (ones col = denom)
    U    = y^T @ v_aug           [500, 272]     f32r
    oa   = U[:, :256] / U[:, 256]; out = mlp2_f32r(oa^T)

exp(x) ~= (SQ_SCALE*x + SQ_BIAS)^2 + POLY_C on |x|<=0.03 (ACT Square;
same activation-table family as Silu, so no table reloads). Masking is
multiplicative {0,1} on y, matching the reference's where(-1e9) +
post-softmax mask exactly.

HW exec time: ~161-167 us/core (PE active ~133 us; ldweights-paced on
the DoubleRow path: DR stationary loads are 256 columns at ~P/1.2 ns
and cannot pipeline under a 60 ns stream, so the fp8 k-path runs at
~213 ns/matmul against f32r's ~120 -- still ahead, plus it frees the
scores matmuls at 241 vs 427 ns/chunk). Baseline f32r kernel: 259 us.
Relative error: 2.28e-4 (the fused fp8 score path contributes ~2e-5;
the rest is the f32r value path, identical to the baseline envelope).
Note: the axon trn2 pool shows ~15-20% device-throttle variance
between invocations; 161.2 us was the best clean measurement.
"""
import numpy as np
from contextlib import ExitStack

import concourse.bass as bass
import concourse.mybir as mybir
import concourse.tile as tile
from concourse import masks
from concourse.bass_utils import run_bass_kernel_spmd

F32 = mybir.dt.float32
F32R = mybir.dt.float32r
F8 = mybir.dt.float8e4
AF = mybir.ActivationFunctionType
ALU = mybir.AluOpType
DR = mybir.MatmulPerfMode.DoubleRow

NCORES = 8
B, N, NE = 64, 512, 500
FD, ED, HID, L = 256, 64, 512, 256
BL = B // NCORES  # local batches per core

SCALE = float(L) ** -0.5
# exp(x) ~= (SQ_SCALE*x + SQ_BIAS)^2 + POLY_C  on [-0.03, 0.03]
SQ_SCALE = 0.7070802649303285
SQ_BIAS = 0.7072128419829565
POLY_C = 0.49985002566041925
WS = 16.0   # fp8 weight prescale
RS = 1.0 / WS

NEP = 512  # padded energy width (div-16 free dims keep the fast path)
LA = 272   # v_aug width: 256 value cols + ones cols (denominator)
CAP0 = 288  # token capacity after mask compaction (18*16, div-16)
CAP1 = 336  # fallback tier (seed-0 max count is 283; binomial tail safety)
E_CHUNKS = [(0, 128), (128, 128), (256, 128), (384, 116)]

# weights loaded straight into f32r tiles (fast-ring, castless)
F32R_PARAMS = {"q_w1", "v_w1", "v_w2", "v_w3", "o_w1", "o_w2",
               "v_b3", "o_b2"}

W_SPECS = [
    ("q_w1", [ED, HID]), ("q_b1", [HID]),
    ("q_w2", [HID, HID]), ("q_b2", [HID]),
    ("q_w3", [HID, L]), ("q_b3", [L]),
    ("k_w1", [FD, HID]), ("k_b1", [HID]),
    ("k_w2", [HID, HID]), ("k_b2", [HID]),
    ("k_w3", [HID, L]), ("k_b3", [L]),
    ("v_w1", [FD, HID]), ("v_b1", [HID]),
    ("v_w2", [HID, HID]), ("v_b2", [HID]),
    ("v_w3", [HID, L]), ("v_b3", [L]),
    ("o_w1", [L, HID]), ("o_b1", [HID]),
    ("o_w2", [HID, L]), ("o_b2", [L]),
]


def split_excess_waits(nc, limit=1):
    """This walrus build rejects >1 sync wait per instruction; move extras
    onto same-engine NoOps inserted immediately before the instruction."""
    for f in nc.m.functions:
        for bb in f.blocks:
            out, changed = [], False
            for inst in bb.instructions:
                si = inst.sync_info
                waits = list(si.on_wait) if si and si.on_wait else []
                if len(waits) > limit:
                    changed = True
                    head, tail = waits[:-limit], waits[-limit:]
                    for j in range(0, len(head), limit):
                        nop = mybir.InstNoOp(
                            name=f"{inst.name}-ws{j}", ins=[], outs=[])
                        nop.engine = inst.engine
                        nop.sync_info = mybir.SyncInfo(
                            on_wait=head[j:j + limit], on_update=[])
                        out.append(nop)
                    inst.sync_info = mybir.SyncInfo(
                        on_wait=tail, on_update=list(si.on_update or []))
                out.append(inst)
            if changed:
                bb.instructions = out


def _build_nc(cap):
    tch = (cap + 127) // 128  # token chunks
    t_chunks = [(i * 128, min(128, cap - i * 128)) for i in range(tch)]

    nc = bass.Bass()
    fldT_d = nc.declare_dram_parameter("fldT", [BL, 2, 128, cap], F32R,
                                       isOutput=False)
    padm_d = nc.declare_dram_parameter("padm", [128, tch, BL], F32,
                                       isOutput=False)
    eT_d = nc.declare_dram_parameter("e_featT", [ED, NEP], F32R,
                                     isOutput=False)
    kw3T_d = nc.declare_dram_parameter("k_w3T", [L, HID], F32,
                                       isOutput=False)
    wd = {nm: nc.declare_dram_parameter(
              nm, shp, F32R if nm in F32R_PARAMS else F32, isOutput=False)
          for nm, shp in W_SPECS}
    ones_d = nc.declare_dram_parameter("ones_in", [128, 128], F32,
                                       isOutput=False)
    out_d = nc.declare_dram_parameter("out", [BL, NE, L], F32, isOutput=True)

    with ExitStack() as ctx:
        tc = ctx.enter_context(tile.TileContext(nc))
        cpool = ctx.enter_context(tc.tile_pool(name="const", bufs=1))
        apool = ctx.enter_context(tc.tile_pool(name="act", bufs=1))
        hpool = ctx.enter_context(tc.tile_pool(name="hid", bufs=2))
        dpool = ctx.enter_context(tc.tile_pool(name="dbuf", bufs=2))
        wst = ctx.enter_context(tc.tile_pool(name="wstage", bufs=2))
        # PSUM: 8 banks total. A-stream MLP psums and B-stream psums get
        # separate pools so the two batch streams never contend.
        ps_mm = ctx.enter_context(
            tc.tile_pool(name="ps_mm", bufs=3, space="PSUM"))
        ps_s = ctx.enter_context(
            tc.tile_pool(name="ps_s", bufs=2, space="PSUM"))
        ps_u = ctx.enter_context(
            tc.tile_pool(name="ps_u", bufs=2, space="PSUM"))
        ps_tp = ctx.enter_context(
            tc.tile_pool(name="ps_tp", bufs=1, space="PSUM"))

        def wchunks(name, rows, cols, eng=None):
            chunks = []
            for c in range(rows // 128):
                t = cpool.tile([128, cols], F32R, name=f"{name}_{c}")
                (eng or nc.sync).dma_start(
                    t[:], wd[name][c * 128:(c + 1) * 128, :])
                chunks.append(t)
            return chunks

        def bias_col(name, ln):
            t = cpool.tile([128, ln // 128], F32, name=f"{name}_col")
            nc.sync.dma_start(t[:], wd[name].rearrange("(c p) -> p c", p=128))
            return t

        def w8pairs(name, rows, cols, eng=None, src=None):
            """fp8 weight pair-tiles [128, 2, cols], prescaled by WS.
            Pair p covers input rows [256p, 256p+256) with dim1 = k-tile.
            All stagings share one [128, 2, 512] slot (2-deep rotation)."""
            outs = []
            for p in range(rows // 256):
                st = wst.tile([128, 2, HID], F32, name="w8_st")
                (eng or nc.sync).dma_start(
                    st[:, :, :cols],
                    (src if src is not None else wd[name])[
                        p * 256:(p + 1) * 256, :].rearrange(
                        "(t p) m -> p t m", t=2))
                t = cpool.tile([128, 2, cols], F8, name=f"{name}_8_{p}")
                nc.vector.tensor_scalar_mul(t[:], st[:, :, :cols], WS)
                outs.append(t)
            return outs

        def load_fld(b):
            fldT = dpool.tile([128, 2, cap], F32R, name="fldT")
            nc.sync.dma_start(fldT[:],
                              fldT_d[b].rearrange("t p c -> p t c"))
            fldT8 = dpool.tile([128, 2, cap], F8, name="fldT8")
            nc.vector.tensor_copy(fldT8[:], fldT[:])
            return fldT, fldT8

        # ---- stage A: k/v MLP of batch f (ACT-heavy) ----
        def a1_kl1(st):
            st["kh1_8"] = kh1_8 = hpool.tile([128, 4, cap], F8, name="kh1_8")
            for oc in range(4):
                pm = ps_mm.tile([128, 512], F32, name="pm_k1", tag="pm")
                nc.tensor.matmul(pm[:, :cap],
                                 kw1_8[0][:, :, oc * 128:(oc + 1) * 128],
                                 st["fldT8"][:], start=True, stop=True,
                                 perf_mode=DR)
                nc.scalar.activation(kh1_8[:, oc, :], pm[:, :cap], AF.Silu,
                                     bias=kb1[:, oc:oc + 1], scale=RS)

        def a2_vl1(st):
            st["vh1"] = vh1 = hpool.tile([128, 4, cap], F32R, name="vh1")
            fldT = st["fldT"]
            for oc in range(4):
                pm = ps_mm.tile([128, 512], F32, name="pm_v1", tag="pm")
                for dc in range(2):
                    nc.tensor.matmul(pm[:, :cap],
                                     vw1[dc][:, oc * 128:(oc + 1) * 128],
                                     fldT[:, dc, :],
                                     start=(dc == 0), stop=(dc == 1))
                nc.scalar.activation(vh1[:, oc, :], pm[:, :cap], AF.Silu,
                                     bias=vb1[:, oc:oc + 1])

        def a3_kl2(st):
            st["kh2_8"] = kh2_8 = hpool.tile([128, 4, cap], F8, name="kh2_8")
            kh1_8 = st["kh1_8"]
            for oc in range(4):
                pm = ps_mm.tile([128, 512], F32, name="pm_k2", tag="pm")
                for p in range(2):
                    nc.tensor.matmul(pm[:, :cap],
                                     kw2_8[p][:, :, oc * 128:(oc + 1) * 128],
                                     kh1_8[:, 2 * p:2 * p + 2, :],
                                     start=(p == 0), stop=(p == 1),
                                     perf_mode=DR)
                nc.scalar.activation(kh2_8[:, oc, :], pm[:, :cap], AF.Silu,
                                     bias=kb2[:, oc:oc + 1], scale=RS)

        def a4_vl2(st):
            st["vh2"] = vh2 = hpool.tile([128, 4, cap], F32R, name="vh2")
            vh1 = st["vh1"]
            for oc in range(4):
                pm = ps_mm.tile([128, 512], F32, name="pm_v2", tag="pm")
                for kc in range(4):
                    nc.tensor.matmul(pm[:, :cap],
                                     vw2[kc][:, oc * 128:(oc + 1) * 128],
                                     vh1[:, kc, :],
                                     start=(kc == 0), stop=(kc == 3))
                nc.scalar.activation(vh2[:, oc, :], pm[:, :cap], AF.Silu,
                                     bias=vb2[:, oc:oc + 1])

        # ---- stage B: scores/attention/out-MLP of batch b ----
        def b1_scores(b, st):
            # k L3 is folded into the scores: s = kh2 . (W3^T q), using the
            # once-per-core qwT8. The dropped q.b3 term is a per-energy
            # constant that cancels in softmax (poly-exp arg shrinks to
            # |x| <= 0.021). The 1/sqrt(L) scale rides the Square act.
            kh2_8 = st["kh2_8"]
            for nch, (off, sz) in enumerate(t_chunks):
                pm = ps_s.tile([128, 512], F32, name="pm_s", tag="ps")
                for p in range(2):
                    nc.tensor.matmul(pm[:sz, :],
                                     kh2_8[:, 2 * p:2 * p + 2, off:off + sz],
                                     qwT8[:, 2 * p:2 * p + 2, :],
                                     start=(p == 0), stop=(p == 1),
                                     perf_mode=DR)
                ytmp = dpool.tile([128, NEP], F32, name="ytmp")
                nc.scalar.activation(ytmp[:sz, :], pm[:sz, :], AF.Square,
                                     bias=sqb_col[:sz], scale=SQ_SCALE * SCALE)
                nc.vector.tensor_scalar(
                    y[:sz, nch, :], ytmp[:sz, :],
                    POLY_C, padm[:sz, nch, b:b + 1],
                    op0=ALU.add, op1=ALU.mult)

        def b2_vl3(st):
            vh2 = st["vh2"]
            for nch, (off, sz) in enumerate(t_chunks):
                pu = ps_u.tile([128, LA], F32, name="pu_v", tag="pu")
                for kc in range(4):
                    nc.tensor.matmul(
                        pu[:sz, :L],
                        vh2[:, kc, off:off + sz],
                        vw3[kc][:], start=(kc == 0), stop=(kc == 3))
                nc.vector.tensor_tensor(
                    v_aug[:sz, nch, :L], pu[:sz, :L], vb3_bc[:sz, :],
                    op=ALU.add)

        def b3_u(st):
            # U = y^T @ [v|1]; normalize into oa (pad tokens have y=0)
            st["oa"] = oa = dpool.tile([128, 4, L], F32R, name="oa")
            for ec, (off, sz) in enumerate(E_CHUNKS):
                pu = ps_u.tile([128, LA], F32, name="pu_a", tag="pu")
                for nch in range(tch):
                    nc.tensor.matmul(pu[:sz, :], y[:, nch, off:off + sz],
                                     v_aug[:, nch, :],
                                     start=(nch == 0), stop=(nch == tch - 1))
                recip = dpool.tile([128, 1], F32, name="recip")
                nc.vector.reciprocal(recip[:sz], pu[:sz, L:L + 1])
                nc.vector.tensor_scalar_mul(oa[:sz, ec, :], pu[:sz, :L],
                                            recip[:sz])

        def b4_oat(st):
            st["oaT"] = oaT = dpool.tile([128, 2, NEP], F32R, name="oaT")
            nc.vector.tensor_copy(
                oaT[:, :, NE:NEP],
                zeros_r[:].rearrange("p (a b) -> p a b", a=2))
            oa = st["oa"]
            for ec, (off, sz) in enumerate(E_CHUNKS):
                # alternate psum pools so chunk ec+1's transposes never
                # wait on chunk ec's DVE drain
                if ec % 2 == 0:
                    pt = ps_tp.tile([128, 2, 128], F32R, name="pt_a",
                                    tag="pt")
                else:
                    pt = ps_s.tile([128, 2, 128], F32R, name="pt_b",
                                   tag="ps")
                for lc in range(2):
                    nc.tensor.transpose(
                        pt[:, lc, :sz], oa[:sz, ec, lc * 128:(lc + 1) * 128],
                        ident_r[:sz, :sz])
                nc.vector.tensor_copy(oaT[:, :, off:off + sz],
                                      pt[:, :, :sz])

        def b5_ol1(st):
            st["oh"] = oh = hpool.tile([128, 4, NEP], F32R, name="oh")
            oaT = st["oaT"]
            for oc in range(4):
                pm = ps_s.tile([128, 512], F32, name="pm_o1", tag="ps")
                for lc in range(2):
                    nc.tensor.matmul(pm[:],
                                     ow1[lc][:, oc * 128:(oc + 1) * 128],
                                     oaT[:, lc, :],
                                     start=(lc == 0), stop=(lc == 1))
                nc.scalar.activation(oh[:, oc, :], pm[:], AF.Silu,
                                     bias=ob1[:, oc:oc + 1])

        def b6_ol2(b, st):
            oh = st["oh"]
            yout = dpool.tile([128, 4, L], F32, name="yout")
            for ec, (off, sz) in enumerate(E_CHUNKS):
                pu = ps_u.tile([128, LA], F32, name="pu_o", tag="pu")
                for hc in range(4):
                    nc.tensor.matmul(pu[:sz, :L], oh[:, hc, off:off + sz],
                                     ow2[hc][:], start=(hc == 0),
                                     stop=(hc == 3))
                nc.vector.tensor_tensor(
                    yout[:sz, ec, :], pu[:sz, :L], ob2_bc[:sz, :],
                    op=ALU.add)
                nc.sync.dma_start(out_d[b, off:off + sz], yout[:sz, ec, :])

        # ---- prologue: interleave weight staging with batch 0's MLP so
        # the PE starts as soon as fldT(0) + the k L1 weights land ----
        st_cur = {}
        # the q L1 weights (128 KB) land well before fldT(0) + the k L1
        # staging (295+512 KB): run q L1 first so the PE starts earliest
        eT = cpool.tile([ED, NEP], F32R, name="eT")
        nc.scalar.dma_start(eT[:], eT_d[:])
        qw1 = cpool.tile([ED, HID], F32R, name="qw1")
        nc.scalar.dma_start(qw1[:], wd["q_w1"][:])
        qb1 = bias_col("q_b1", HID)
        # fldT(0) rides sync while the k L1 staging rides scalar: the two
        # transfers that gate the first k matmul run on parallel rings
        st_cur["fldT"], st_cur["fldT8"] = load_fld(0)
        padm = cpool.tile([128, tch, BL], F32, name="padm")
        nc.sync.dma_start(padm[:], padm_d[:])
        kw1_8 = w8pairs("k_w1", FD, HID, eng=nc.scalar)
        kb1 = bias_col("k_b1", HID)
        qh1_8 = apool.tile([128, 4, NEP], F8, name="qh1_8")
        for oc in range(4):
            pm = ps_mm.tile([128, 512], F32, name="pm_q1", tag="pm")
            nc.tensor.matmul(pm[:], qw1[:, oc * 128:(oc + 1) * 128],
                             eT[:], start=True, stop=True)
            nc.scalar.activation(qh1_8[:, oc, :], pm[:], AF.Silu,
                                 bias=qb1[:, oc:oc + 1])
        vw1 = wchunks("v_w1", FD, HID)  # sync: lands right after fldT(0)
        a1_kl1(st_cur)
        vb1 = bias_col("v_b1", HID)
        a2_vl1(st_cur)
        kw2_8 = w8pairs("k_w2", HID, HID, eng=nc.scalar)
        kb2 = bias_col("k_b2", HID)
        a3_kl2(st_cur)
        vw2 = wchunks("v_w2", HID, HID)
        vb2 = bias_col("v_b2", HID)
        qb2 = bias_col("q_b2", HID)
        qb3 = bias_col("q_b3", L)
        qw2_8 = w8pairs("q_w2", HID, HID)
        a4_vl2(st_cur)
        qh2_8 = apool.tile([128, 4, NEP], F8, name="qh2_8")
        for oc in range(4):
            pm = ps_mm.tile([128, 512], F32, name="pm_q2", tag="pm")
            for p in range(2):
                nc.tensor.matmul(pm[:],
                                 qw2_8[p][:, :, oc * 128:(oc + 1) * 128],
                                 qh1_8[:, 2 * p:2 * p + 2, :],
                                 start=(p == 0), stop=(p == 1), perf_mode=DR)
            nc.scalar.activation(qh2_8[:, oc, :], pm[:], AF.Silu,
                                 bias=qb2[:, oc:oc + 1], scale=RS)
        qw3_8 = w8pairs("q_w3", HID, L)
        kw3T_8 = w8pairs("k_w3T", L, HID, src=kw3T_d)
        sqb_col = cpool.tile([128, 1], F32, name="sqb_col")
        nc.gpsimd.memset(sqb_col[:], SQ_BIAS)
        # the q L3 / qwT casts ride DVE, not ACT: the prologue's scalar
        # queue is deep in silus and would stall batch 0's y otherwise
        qTs8 = cpool.tile([128, 2, NEP], F8, name="qTs8")
        for lc in range(2):
            pm = ps_mm.tile([128, 512], F32, name="pm_q3", tag="pm")
            for p in range(2):
                nc.tensor.matmul(pm[:],
                                 qw3_8[p][:, :, lc * 128:(lc + 1) * 128],
                                 qh2_8[:, 2 * p:2 * p + 2, :],
                                 start=(p == 0), stop=(p == 1), perf_mode=DR)
            nc.vector.tensor_scalar(qTs8[:, lc, :], pm[:],
                                    RS, qb3[:, lc:lc + 1],
                                    op0=ALU.mult, op1=ALU.add)
        # qwT8 = W3_k^T q  [h, e]: lets scores contract kh2 directly, so
        # the per-batch k L3 stage (and its kT8 cast) disappear entirely
        qwT8 = cpool.tile([128, 4, NEP], F8, name="qwT8")
        for hc in range(4):
            pm = ps_mm.tile([128, 512], F32, name="pm_qw", tag="pm")
            nc.tensor.matmul(pm[:], kw3T_8[0][:, :, hc * 128:(hc + 1) * 128],
                             qTs8[:], start=True, stop=True, perf_mode=DR)
            nc.vector.tensor_scalar_mul(qwT8[:, hc, :], pm[:], RS)

        # ---- remaining value/out-path weights + consts (sync ring) ----
        vw3 = wchunks("v_w3", HID, L)
        vb3_row = cpool.tile([1, L], F32R, name="vb3_row")
        nc.sync.dma_start(
            vb3_row[:], wd["v_b3"].rearrange("(a n) -> a n", a=1))
        ow1 = wchunks("o_w1", L, HID)
        ob1 = bias_col("o_b1", HID)
        ow2 = wchunks("o_w2", HID, L)
        ob2_row = cpool.tile([1, L], F32R, name="ob2_row")
        nc.sync.dma_start(
            ob2_row[:], wd["o_b2"].rearrange("(a n) -> a n", a=1))

        ident = cpool.tile([128, 128], F32, name="ident")
        masks.make_identity(nc, ident[:])
        ident_r = cpool.tile([128, 128], F32R, name="ident_r")
        nc.vector.tensor_copy(ident_r[:], ident[:])
        zeros_r = cpool.tile([128, 24], F32R, name="zeros_r")
        nc.vector.tensor_scalar_mul(zeros_r[:], ident[:, :24], 0.0)
        ones_blk = cpool.tile([128, 128], F32, name="ones_blk")
        nc.sync.dma_start(ones_blk[:], ones_d[:])
        ones_row = cpool.tile([1, 128], F32R, name="ones_row")
        nc.vector.tensor_copy(ones_row[:], ones_blk[:1, :])

        # bias broadcast tiles [128, 256] (one rank-1 matmul each)
        vb3_bc = cpool.tile([128, L], F32, name="vb3_bc")
        ob2_bc = cpool.tile([128, L], F32, name="ob2_bc")
        pbc = ps_u.tile([128, LA], F32, name="pbc", tag="pu")
        nc.tensor.matmul(pbc[:, :L], ones_row[:, :128], vb3_row[:],
                         start=True, stop=True)
        nc.vector.tensor_copy(vb3_bc[:], pbc[:, :L])
        pbc2 = ps_u.tile([128, LA], F32, name="pbc2", tag="pu")
        nc.tensor.matmul(pbc2[:, :L], ones_row[:, :128], ob2_row[:],
                         start=True, stop=True)
        nc.vector.tensor_copy(ob2_bc[:], pbc2[:, :L])

        # persistent per-batch tiles; zero the never-written pad rows of
        # the last token chunk once (y=0 there kills pad tokens in U)
        y = apool.tile([128, tch, NEP], F32R, name="y")
        v_aug = apool.tile([128, tch, LA], F32R, name="v_aug")
        last_off, last_sz = t_chunks[-1]
        if last_sz < 128:
            zsrc = cpool.tile([128, NEP], F32, name="zsrc")
            nc.gpsimd.memset(zsrc[:], 0.0)
            nc.vector.tensor_copy(y[:, tch - 1, :], zsrc[:, :])
            nc.vector.tensor_copy(v_aug[:, tch - 1, :], zsrc[:, :LA])
        nc.vector.tensor_copy(
            v_aug[:, :, L:LA],
            ones_blk[:, :tch * (LA - L)].rearrange(
                "p (a b) -> p a b", a=tch))

        def b_tail_fine(b, st):
            # final batch: no A-stream left to interleave, so pipeline the
            # B phases at chunk granularity (per-chunk oa/oh tiles avoid
            # whole-tile barriers between U -> norm -> transpose -> o MLP)
            oaT = dpool.tile([128, 2, NEP], F32R, name="oaT")
            nc.vector.tensor_copy(
                oaT[:, :, NE:NEP],
                zeros_r[:].rearrange("p (a b) -> p a b", a=2))
            for ec, (off, sz) in enumerate(E_CHUNKS):
                pu = ps_u.tile([128, LA], F32, name="pu_a", tag="pu")
                for nch in range(tch):
                    nc.tensor.matmul(pu[:sz, :], y[:, nch, off:off + sz],
                                     v_aug[:, nch, :],
                                     start=(nch == 0), stop=(nch == tch - 1))
                recip = dpool.tile([128, 1], F32, name="recip")
                nc.vector.reciprocal(recip[:sz], pu[:sz, L:L + 1])
                oa_c = dpool.tile([128, L], F32R, name=f"oa_c{ec}")
                nc.vector.tensor_scalar_mul(oa_c[:sz, :], pu[:sz, :L],
                                            recip[:sz])
                if ec % 2 == 0:
                    pt = ps_tp.tile([128, 2, 128], F32R, name="pt_a",
                                    tag="pt")
                else:
                    pt = ps_s.tile([128, 2, 128], F32R, name="pt_b",
                                   tag="ps")
                for lc in range(2):
                    nc.tensor.transpose(
                        pt[:, lc, :sz], oa_c[:sz, lc * 128:(lc + 1) * 128],
                        ident_r[:sz, :sz])
                nc.vector.tensor_copy(oaT[:, :, off:off + sz],
                                      pt[:, :, :sz])
            oh_c = []
            for oc in range(4):
                pm = ps_s.tile([128, 512], F32, name="pm_o1", tag="ps")
                for lc in range(2):
                    nc.tensor.matmul(pm[:],
                                     ow1[lc][:, oc * 128:(oc + 1) * 128],
                                     oaT[:, lc, :],
                                     start=(lc == 0), stop=(lc == 1))
                t = hpool.tile([128, NEP], F32R, name=f"oh_c{oc}")
                nc.scalar.activation(t[:], pm[:], AF.Silu,
                                     bias=ob1[:, oc:oc + 1])
                oh_c.append(t)
            yout = dpool.tile([128, 4, L], F32, name="yout")
            for pair in ((0, 1), (2, 3)):
                pus = [ps_u.tile([128, LA], F32, name="pu_o", tag="pu")
                       for _ in pair]
                for hc in range(4):
                    for j, ec in enumerate(pair):
                        off, sz = E_CHUNKS[ec]
                        nc.tensor.matmul(pus[j][:sz, :L],
                                         oh_c[hc][:, off:off + sz],
                                         ow2[hc][:], start=(hc == 0),
                                         stop=(hc == 3))
                for j, ec in enumerate(pair):
                    off, sz = E_CHUNKS[ec]
                    nc.vector.tensor_tensor(yout[:sz, ec, :],
                                            pus[j][:sz, :L],
                                            ob2_bc[:sz, :], op=ALU.add)
                    nc.sync.dma_start(out_d[b, off:off + sz],
                                      yout[:sz, ec, :])

        # ---- skewed main loop: B(b) interleaved with A(b+1); the next
        # batch's scores + v L3 are hoisted to the iteration tail so no
        # iteration starts on a serial scores->y->U or vh2->v_aug chain,
        # and the final (A-less) iteration runs a chunk-fused pipeline ----
        b1_scores(0, st_cur)
        b2_vl3(st_cur)
        for b in range(BL):
            st, st_nxt = st_cur, {}
            if b + 1 >= BL:
                b_tail_fine(b, st)
                break
            st_nxt["fldT"], st_nxt["fldT8"] = load_fld(b + 1)
            b3_u(st)
            a1_kl1(st_nxt)
            b4_oat(st)
            a2_vl1(st_nxt)
            b5_ol1(st)
            a3_kl2(st_nxt)
            a4_vl2(st_nxt)
            b6_ol2(b, st)
            b1_scores(b + 1, st_nxt)
            b2_vl3(st_nxt)
            st_cur = st_nxt

    split_excess_waits(nc)
    return nc


_NC_CACHE = {}


def _get_nc(cap=CAP0):
    if cap not in _NC_CACHE:
        _NC_CACHE[cap] = _build_nc(cap)
    return _NC_CACHE[cap]


def _prep(inputs):
    """Host-side shard prep: mask compaction + layout transposes only."""
    field = np.ascontiguousarray(inputs["field_atom_lat"], dtype=np.float32)
    mask = np.asarray(inputs["mask"]).astype(bool)
    cnts = mask.sum(axis=1)
    cmax = cnts.max()
    cap = CAP0 if cmax <= CAP0 else (CAP1 if cmax <= CAP1 else N)
    tch = (cap + 127) // 128

    idx = np.zeros((B, cap), np.int64)
    for b in range(B):
        nz = np.flatnonzero(mask[b])
        idx[b, :nz.size] = nz
    gathered = field[np.arange(B)[:, None], idx]          # [B, cap, FD]
    fldT = np.ascontiguousarray(
        gathered.transpose(0, 2, 1)).reshape(B, 2, 128, cap)

    padm = np.zeros((B, tch * 128), np.float32)
    padm[:, :cap] = (np.arange(cap)[None, :] < cnts[:, None])
    padm_cols = np.ascontiguousarray(
        padm.reshape(B, tch, 128).transpose(2, 1, 0))     # [128, tch, B]

    eT = np.zeros((ED, NEP), np.float32)
    eT[:, :NE] = np.ascontiguousarray(
        inputs["e_feat"], dtype=np.float32).T
    return cap, fldT, padm_cols, eT


def _kw3T(inputs):
    return np.ascontiguousarray(
        np.asarray(inputs["k_w3"], dtype=np.float32).T)


def _make_in_maps(inputs, fldT, padm_cols, eT):
    in_maps = []
    for c in range(NCORES):
        m = {
            "fldT": fldT[c * BL:(c + 1) * BL],
            "padm": np.ascontiguousarray(
                padm_cols[:, :, c * BL:(c + 1) * BL]),
            "e_featT": eT,
            "k_w3T": _kw3T(inputs),
        }
        for nm, _ in W_SPECS:
            m[nm] = np.ascontiguousarray(inputs[nm], dtype=np.float32)
        m["ones_in"] = np.ones((128, 128), dtype=np.float32)
        in_maps.append(m)
    return in_maps


def kernel(**inputs):
    cap, fldT, padm_cols, eT = _prep(inputs)
    nc = _get_nc(cap)
    in_maps = _make_in_maps(inputs, fldT, padm_cols, eT)
    res = run_bass_kernel_spmd(nc, in_maps, list(range(NCORES)))
    out = np.concatenate([res.results[c]["out"] for c in range(NCORES)],
                         axis=0)
    return out.astype(np.float32)


# revision 43
# speedup vs baseline: 1.0182x; 1.0182x over previous
"""Trainium2 Bass kernel: EnergyConditionedFieldAttention.

Sharding: data-parallel over batch B=64 across 8 NeuronCores (8 batches
per core). MLP weights and the shared query path q = mlp3(e_feat) are
replicated on every core; each core returns out[8, 500, 256] and the
host concatenates.

Structural choices over the f32r baseline (259 us):

1. Token compaction. The mask is ~Bernoulli(0.5) per token and masked
   tokens are dead in every term (scores -1e9 AND post-softmax mask),
   so the host gathers unmasked field rows per batch into CAP=336
   columns (mean count 256, sigma 11.3 -> 7 sigma headroom; if a mask
   ever exceeds CAP the kernel transparently rebuilds at 512). A 0/1
   pad-mask column replaces the token mask on-device. The host also
   pre-transposes the gathered field to [2, 128, CAP] (layout-only
   work, like the per-core batch slicing), killing the per-batch PE
   transposes.

2. fp8 (e4m3) DoubleRow matmuls on the score path. DoubleRow contracts
   two 128-row k-tiles per pass at ~4x f32r row rate. Scaled scores
   satisfy |s| <= ~0.03, so relative error eps in q/k perturbs
   attention weights by only ~eps*|s|: the q-MLP (L2/L3), k-MLP and
   scores run in fp8 adding just ~2e-5 output error (measured against
   an exact numpy emulation). Weights are prescaled x16 on-device
   before the fp8 cast (uniform-init weights sit below the e4m3
   normal range; x16 lifts them to [0.25, 1] clear of any FTZ), and
   the inverse 1/16 rides the next activation's scale port. The value
   path (v-MLP, U, out-MLP) stays f32r: attention averaging does not
   suppress v-side noise (fp8 there costs ~3% output error).

3. One-batch software skew. Stage A(b+1) = k/v MLP (ACT-heavy silu
   chains) is interleaved phase-by-phase into stage B(b) = scores/
   attention/out-MLP (PE/DVE-heavy). Separate PSUM pools per stream
   (ps_mm vs ps_s/ps_u) keep a silu drain from ever stalling the PE:
   it just runs the other stream's matmuls.

4. All f32r weights/field are declared float32r in DRAM (same 32 bits,
   numpy float32 on host) so they ride the two fast HWDGE rings (sync
   + scalar) with no SWDGE cast; staging for the fp8 casts interleaves
   with batch-0's MLP so the PE starts ~3 us into the kernel.

Per-core plan (PSUM accumulation fp32 throughout):
  qTs8 = mlp3_fp8(e_featT)       [128, 2, NEP]  once; raw q, fp8
  qwT8 = W3_k^T qTs8             [128, 4, NEP]  once; k L3 folded into
         scores: s = kh2 . qwT. The dropped q.b3 term is a per-energy
         constant that cancels in softmax; the poly-exp argument
         actually shrinks (|x| <= 0.021). Kills the per-batch k L3
         matmuls AND the kT8 fp8 cast on the iteration-tail chain.
  per local batch b:
    kh*  = fp8 silu chain        [128, 4, CAP]  DoubleRow
    sT   = kh2_8^T (x) qwT8      [tok, NEP]     DoubleRow, raw scores
    y    = ((c1*S*s+c2)^2+c3)*padmask           ACT Square + DVE
    v    = f32r mlp3             v_aug [128, 3, 272] (ones col = denom)
    U    = y^T @ v_aug           [500, 272]     f32r
    oa   = U[:, :256] / U[:, 256]; out = mlp2_f32r(oa^T)

exp(x) ~= (SQ_SCALE*x + SQ_BIAS)^2 + POLY_C on |x|<=0.03 (ACT Square;
same activation-table family as Silu, so no table reloads). Masking is
multiplicative {0,1} on y, matching the reference's where(-1e9) +
post-softmax mask exactly.

HW exec time: ~161-167 us/core (PE active ~133 us; ldweights-paced on
the DoubleRow path: DR stationary loads are 256 columns at ~P/1.2 ns
and cannot pipeline under a 60 ns stream, so the fp8 k-path runs at
~213 ns/matmul against f32r's ~120 -- still ahead, plus it frees the
scores matmuls at 241 vs 427 ns/chunk). Baseline f32r kernel: 259 us.
Relative error: 2.28e-4 (the fused fp8 score path contributes ~2e-5;
the rest is the f32r value path, identical to the baseline envelope).
Note: the axon trn2 pool shows ~15-20% device-throttle variance
between invocations; 161.2 us was the best clean measurement.
"""
import numpy as np
from contextlib import ExitStack

import concourse.bass as bass
import concourse.mybir as mybir
import concourse.tile as tile
from concourse import masks
from concourse.bass_utils import run_bass_kernel_spmd

F32 = mybir.dt.float32
F32R = mybir.dt.float32r
F8 = mybir.dt.float8e4
AF = mybir.ActivationFunctionType
ALU = mybir.AluOpType
DR = mybir.MatmulPerfMode.DoubleRow

NCORES = 8
B, N, NE = 64, 512, 500
FD, ED, HID, L = 256, 64, 512, 256
BL = B // NCORES  # local batches per core

SCALE = float(L) ** -0.5
# exp(x) ~= (SQ_SCALE*x + SQ_BIAS)^2 + POLY_C  on [-0.03, 0.03]
SQ_SCALE = 0.7070802649303285
SQ_BIAS = 0.7072128419829565
POLY_C = 0.49985002566041925
WS = 16.0   # fp8 weight prescale
RS = 1.0 / WS

NEP = 512  # padded energy width (div-16 free dims keep the fast path)
LA = 272   # v_aug width: 256 value cols + ones cols (denominator)
CAP0 = 288  # token capacity after mask compaction (18*16, div-16)
CAP1 = 336  # fallback tier (seed-0 max count is 283; binomial tail safety)
E_CHUNKS = [(0, 128), (128, 128), (256, 128), (384, 116)]

# weights loaded straight into f32r tiles (fast-ring, castless)
F32R_PARAMS = {"q_w1", "v_w1", "v_w2", "v_w3", "o_w1", "o_w2",
               "v_b3", "o_b2"}

W_SPECS = [
    ("q_w1", [ED, HID]), ("q_b1", [HID]),
    ("q_w2", [HID, HID]), ("q_b2", [HID]),
    ("q_w3", [HID, L]), ("q_b3", [L]),
    ("k_w1", [FD, HID]), ("k_b1", [HID]),
    ("k_w2", [HID, HID]), ("k_b2", [HID]),
    ("k_w3", [HID, L]), ("k_b3", [L]),
    ("v_w1", [FD, HID]), ("v_b1", [HID]),
    ("v_w2", [HID, HID]), ("v_b2", [HID]),
    ("v_w3", [HID, L]), ("v_b3", [L]),
    ("o_w1", [L, HID]), ("o_b1", [HID]),
    ("o_w2", [HID, L]), ("o_b2", [L]),
]


def split_excess_waits(nc, limit=1):
    """This walrus build rejects >1 sync wait per instruction; move extras
    onto same-engine NoOps inserted immediately before the instruction."""
    for f in nc.m.functions:
        for bb in f.blocks:
            out, changed = [], False
            for inst in bb.instructions:
                si = inst.sync_info
                waits = list(si.on_wait) if si and si.on_wait else []
                if len(waits) > limit:
                    changed = True
                    head, tail = waits[:-limit], waits[-limit:]
                    for j in range(0, len(head), limit):
                        nop = mybir.InstNoOp(
                            name=f"{inst.name}-ws{j}", ins=[], outs=[])
                        nop.engine = inst.engine
                        nop.sync_info = mybir.SyncInfo(
                            on_wait=head[j:j + limit], on_update=[])
                        out.append(nop)
                    inst.sync_info = mybir.SyncInfo(
                        on_wait=tail, on_update=list(si.on_update or []))
                out.append(inst)
            if changed:
                bb.instructions = out


def _build_nc(cap):
    tch = (cap + 127) // 128  # token chunks
    t_chunks = [(i * 128, min(128, cap - i * 128)) for i in range(tch)]

    nc = bass.Bass()
    fldT_d = nc.declare_dram_parameter("fldT", [BL, 2, 128, cap], F32R,
                                       isOutput=False)
    padm_d = nc.declare_dram_parameter("padm", [128, tch, BL], F32,
                                       isOutput=False)
    eT_d = nc.declare_dram_parameter("e_featT", [ED, NEP], F32R,
                                     isOutput=False)
    kw3T_d = nc.declare_dram_parameter("k_w3T", [L, HID], F32,
                                       isOutput=False)
    wd = {nm: nc.declare_dram_parameter(
              nm, shp, F32R if nm in F32R_PARAMS else F32, isOutput=False)
          for nm, shp in W_SPECS}
    ones_d = nc.declare_dram_parameter("ones_in", [128, 128], F32,
                                       isOutput=False)
    out_d = nc.declare_dram_parameter("out", [BL, NE, L], F32, isOutput=True)

    with ExitStack() as ctx:
        tc = ctx.enter_context(tile.TileContext(nc))
        cpool = ctx.enter_context(tc.tile_pool(name="const", bufs=1))
        apool = ctx.enter_context(tc.tile_pool(name="act", bufs=1))
        hpool = ctx.enter_context(tc.tile_pool(name="hid", bufs=2))
        dpool = ctx.enter_context(tc.tile_pool(name="dbuf", bufs=2))
        wst = ctx.enter_context(tc.tile_pool(name="wstage", bufs=2))
        # PSUM: 8 banks total. A-stream MLP psums and B-stream psums get
        # separate pools so the two batch streams never contend.
        ps_mm = ctx.enter_context(
            tc.tile_pool(name="ps_mm", bufs=3, space="PSUM"))
        ps_s = ctx.enter_context(
            tc.tile_pool(name="ps_s", bufs=2, space="PSUM"))
        ps_u = ctx.enter_context(
            tc.tile_pool(name="ps_u", bufs=2, space="PSUM"))
        ps_tp = ctx.enter_context(
            tc.tile_pool(name="ps_tp", bufs=1, space="PSUM"))

        def wchunks(name, rows, cols, eng=None):
            chunks = []
            for c in range(rows // 128):
                t = cpool.tile([128, cols], F32R, name=f"{name}_{c}")
                (eng or nc.sync).dma_start(
                    t[:], wd[name][c * 128:(c + 1) * 128, :])
                chunks.append(t)
            return chunks

        def bias_col(name, ln):
            t = cpool.tile([128, ln // 128], F32, name=f"{name}_col")
            nc.sync.dma_start(t[:], wd[name].rearrange("(c p) -> p c", p=128))
            return t

        def w8pairs(name, rows, cols, eng=None, src=None):
            """fp8 weight pair-tiles [128, 2, cols], prescaled by WS.
            Pair p covers input rows [256p, 256p+256) with dim1 = k-tile.
            All stagings share one [128, 2, 512] slot (2-deep rotation)."""
            outs = []
            for p in range(rows // 256):
                st = wst.tile([128, 2, HID], F32, name="w8_st")
                (eng or nc.sync).dma_start(
                    st[:, :, :cols],
                    (src if src is not None else wd[name])[
                        p * 256:(p + 1) * 256, :].rearrange(
                        "(t p) m -> p t m", t=2))
                t = cpool.tile([128, 2, cols], F8, name=f"{name}_8_{p}")
                nc.vector.tensor_scalar_mul(t[:], st[:, :, :cols], WS)
                outs.append(t)
            return outs

        def load_fld(b):
            fldT = dpool.tile([128, 2, cap], F32R, name="fldT")
            nc.sync.dma_start(fldT[:],
                              fldT_d[b].rearrange("t p c -> p t c"))
            fldT8 = dpool.tile([128, 2, cap], F8, name="fldT8")
            nc.vector.tensor_copy(fldT8[:], fldT[:])
            return fldT, fldT8

        # ---- stage A: k/v MLP of batch f (ACT-heavy) ----
        def a1_kl1(st):
            st["kh1_8"] = kh1_8 = hpool.tile([128, 4, cap], F8, name="kh1_8")
            for oc in range(4):
                pm = ps_mm.tile([128, 512], F32, name="pm_k1", tag="pm")
                nc.tensor.matmul(pm[:, :cap],
                                 kw1_8[0][:, :, oc * 128:(oc + 1) * 128],
                                 st["fldT8"][:], start=True, stop=True,
                                 perf_mode=DR)
                nc.scalar.activation(kh1_8[:, oc, :], pm[:, :cap], AF.Silu,
                                     bias=kb1[:, oc:oc + 1], scale=RS)

        def a2_vl1(st):
            st["vh1"] = vh1 = hpool.tile([128, 4, cap], F32R, name="vh1")
            fldT = st["fldT"]
            for oc in range(4):
                pm = ps_mm.tile([128, 512], F32, name="pm_v1", tag="pm")
                for dc in range(2):
                    nc.tensor.matmul(pm[:, :cap],
                                     vw1[dc][:, oc * 128:(oc + 1) * 128],
                                     fldT[:, dc, :],
                                     start=(dc == 0), stop=(dc == 1))
                nc.scalar.activation(vh1[:, oc, :], pm[:, :cap], AF.Silu,
                                     bias=vb1[:, oc:oc + 1])

        def a3_kl2(st):
            st["kh2_8"] = kh2_8 = hpool.tile([128, 4, cap], F8, name="kh2_8")
            kh1_8 = st["kh1_8"]
            for oc in range(4):
                pm = ps_mm.tile([128, 512], F32, name="pm_k2", tag="pm")
                for p in range(2):
                    nc.tensor.matmul(pm[:, :cap],
                                     kw2_8[p][:, :, oc * 128:(oc + 1) * 128],
                                     kh1_8[:, 2 * p:2 * p + 2, :],
                                     start=(p == 0), stop=(p == 1),
                                     perf_mode=DR)
                nc.scalar.activation(kh2_8[:, oc, :], pm[:, :cap], AF.Silu,
                                     bias=kb2[:, oc:oc + 1], scale=RS)

        def a4_vl2(st):
            st["vh2"] = vh2 = hpool.tile([128, 4, cap], F32R, name="vh2")
            vh1 = st["vh1"]
            for oc in range(4):
                pm = ps_mm.tile([128, 512], F32, name="pm_v2", tag="pm")
                for kc in range(4):
                    nc.tensor.matmul(pm[:, :cap],
                                     vw2[kc][:, oc * 128:(oc + 1) * 128],
                                     vh1[:, kc, :],
                                     start=(kc == 0), stop=(kc == 3))
                nc.scalar.activation(vh2[:, oc, :], pm[:, :cap], AF.Silu,
                                     bias=vb2[:, oc:oc + 1])

        # ---- stage B: scores/attention/out-MLP of batch b ----
        def b1_scores(b, st):
            # k L3 is folded into the scores: s = kh2 . (W3^T q), using the
            # once-per-core qwT8. The dropped q.b3 term is a per-energy
            # constant that cancels in softmax (poly-exp arg shrinks to
            # |x| <= 0.021). The 1/sqrt(L) scale rides the Square act.
            kh2_8 = st["kh2_8"]
            for nch, (off, sz) in enumerate(t_chunks):
                pm = ps_s.tile([128, 512], F32, name="pm_s", tag="ps")
                for p in range(2):
                    nc.tensor.matmul(pm[:sz, :],
                                     kh2_8[:, 2 * p:2 * p + 2, off:off + sz],
                                     qwT8[:, 2 * p:2 * p + 2, :],
                                     start=(p == 0), stop=(p == 1),
                                     perf_mode=DR)
                ytmp = dpool.tile([128, NEP], F32, name="ytmp")
                nc.scalar.activation(ytmp[:sz, :], pm[:sz, :], AF.Square,
                                     bias=sqb_col[:sz], scale=SQ_SCALE * SCALE)
                nc.vector.tensor_scalar(
                    y[:sz, nch, :], ytmp[:sz, :],
                    POLY_C, padm[:sz, nch, b:b + 1],
                    op0=ALU.add, op1=ALU.mult)

        def b2_vl3(st):
            vh2 = st["vh2"]
            for nch, (off, sz) in enumerate(t_chunks):
                pu = ps_u.tile([128, LA], F32, name="pu_v", tag="pu")
                for kc in range(4):
                    nc.tensor.matmul(
                        pu[:sz, :L],
                        vh2[:, kc, off:off + sz],
                        vw3[kc][:], start=(kc == 0), stop=(kc == 3))
                nc.vector.tensor_tensor(
                    v_aug[:sz, nch, :L], pu[:sz, :L], vb3_bc[:sz, :],
                    op=ALU.add)

        def b3_u(st):
            # U = y^T @ [v|1]; normalize into oa (pad tokens have y=0)
            st["oa"] = oa = dpool.tile([128, 4, L], F32R, name="oa")
            for ec, (off, sz) in enumerate(E_CHUNKS):
                pu = ps_u.tile([128, LA], F32, name="pu_a", tag="pu")
                for nch in range(tch):
                    nc.tensor.matmul(pu[:sz, :], y[:, nch, off:off + sz],
                                     v_aug[:, nch, :],
                                     start=(nch == 0), stop=(nch == tch - 1))
                recip = dpool.tile([128, 1], F32, name="recip")
                nc.vector.reciprocal(recip[:sz], pu[:sz, L:L + 1])
                nc.vector.tensor_scalar_mul(oa[:sz, ec, :], pu[:sz, :L],
                                            recip[:sz])

        def b4_oat(st):
            st["oaT"] = oaT = dpool.tile([128, 2, NEP], F32R, name="oaT")
            nc.vector.tensor_copy(
                oaT[:, :, NE:NEP],
                zeros_r[:].rearrange("p (a b) -> p a b", a=2))
            oa = st["oa"]
            for ec, (off, sz) in enumerate(E_CHUNKS):
                # alternate psum pools so chunk ec+1's transposes never
                # wait on chunk ec's DVE drain
                if ec % 2 == 0:
                    pt = ps_tp.tile([128, 2, 128], F32R, name="pt_a",
                                    tag="pt")
                else:
                    pt = ps_s.tile([128, 2, 128], F32R, name="pt_b",
                                   tag="ps")
                for lc in range(2):
                    nc.tensor.transpose(
                        pt[:, lc, :sz], oa[:sz, ec, lc * 128:(lc + 1) * 128],
                        ident_r[:sz, :sz])
                nc.vector.tensor_copy(oaT[:, :, off:off + sz],
                                      pt[:, :, :sz])

        def b5_ol1(st):
            st["oh"] = oh = hpool.tile([128, 4, NEP], F32R, name="oh")
            oaT = st["oaT"]
            for oc in range(4):
                pm = ps_s.tile([128, 512], F32, name="pm_o1", tag="ps")
                for lc in range(2):
                    nc.tensor.matmul(pm[:],
                                     ow1[lc][:, oc * 128:(oc + 1) * 128],
                                     oaT[:, lc, :],
                                     start=(lc == 0), stop=(lc == 1))
                nc.scalar.activation(oh[:, oc, :], pm[:], AF.Silu,
                                     bias=ob1[:, oc:oc + 1])

        def b6_ol2(b, st):
            oh = st["oh"]
            yout = dpool.tile([128, 4, L], F32, name="yout")
            for ec, (off, sz) in enumerate(E_CHUNKS):
                pu = ps_u.tile([128, LA], F32, name="pu_o", tag="pu")
                for hc in range(4):
                    nc.tensor.matmul(pu[:sz, :L], oh[:, hc, off:off + sz],
                                     ow2[hc][:], start=(hc == 0),
                                     stop=(hc == 3))
                nc.vector.tensor_tensor(
                    yout[:sz, ec, :], pu[:sz, :L], ob2_bc[:sz, :],
                    op=ALU.add)
                nc.sync.dma_start(out_d[b, off:off + sz], yout[:sz, ec, :])

        # ---- prologue: interleave weight staging with batch 0's MLP so
        # the PE starts as soon as fldT(0) + the k L1 weights land ----
        st_cur = {}
        # the q L1 weights (128 KB) land well before fldT(0) + the k L1
        # staging (295+512 KB): run q L1 first so the PE starts earliest
        eT = cpool.tile([ED, NEP], F32R, name="eT")
        nc.scalar.dma_start(eT[:], eT_d[:])
        qw1 = cpool.tile([ED, HID], F32R, name="qw1")
        nc.scalar.dma_start(qw1[:], wd["q_w1"][:])
        qb1 = bias_col("q_b1", HID)
        # fldT(0) rides sync while the k L1 staging rides scalar: the two
        # transfers that gate the first k matmul run on parallel rings
        st_cur["fldT"], st_cur["fldT8"] = load_fld(0)
        padm = cpool.tile([128, tch, BL], F32, name="padm")
        nc.sync.dma_start(padm[:], padm_d[:])
        kw1_8 = w8pairs("k_w1", FD, HID, eng=nc.scalar)
        kb1 = bias_col("k_b1", HID)
        qh1_8 = apool.tile([128, 4, NEP], F8, name="qh1_8")
        for oc in range(4):
            pm = ps_mm.tile([128, 512], F32, name="pm_q1", tag="pm")
            nc.tensor.matmul(pm[:], qw1[:, oc * 128:(oc + 1) * 128],
                             eT[:], start=True, stop=True)
            nc.scalar.activation(qh1_8[:, oc, :], pm[:], AF.Silu,
                                 bias=qb1[:, oc:oc + 1])
        vw1 = wchunks("v_w1", FD, HID)  # sync: lands right after fldT(0)
        a1_kl1(st_cur)
        vb1 = bias_col("v_b1", HID)
        a2_vl1(st_cur)
        kw2_8 = w8pairs("k_w2", HID, HID, eng=nc.scalar)
        kb2 = bias_col("k_b2", HID)
        a3_kl2(st_cur)
        vw2 = wchunks("v_w2", HID, HID)
        vb2 = bias_col("v_b2", HID)
        qb2 = bias_col("q_b2", HID)
        qb3 = bias_col("q_b3", L)
        qw2_8 = w8pairs("q_w2", HID, HID)
        a4_vl2(st_cur)
        qh2_8 = apool.tile([128, 4, NEP], F8, name="qh2_8")
        for oc in range(4):
            pm = ps_mm.tile([128, 512], F32, name="pm_q2", tag="pm")
            for p in range(2):
                nc.tensor.matmul(pm[:],
                                 qw2_8[p][:, :, oc * 128:(oc + 1) * 128],
                                 qh1_8[:, 2 * p:2 * p + 2, :],
                                 start=(p == 0), stop=(p == 1), perf_mode=DR)
            nc.scalar.activation(qh2_8[:, oc, :], pm[:], AF.Silu,
                                 bias=qb2[:, oc:oc + 1], scale=RS)
        qw3_8 = w8pairs("q_w3", HID, L, eng=nc.scalar)
        kw3T_8 = w8pairs("k_w3T", L, HID, eng=nc.scalar, src=kw3T_d)
        sqb_col = cpool.tile([128, 1], F32, name="sqb_col")
        nc.gpsimd.memset(sqb_col[:], SQ_BIAS)
        # the q L3 / qwT casts ride DVE, not ACT: the prologue's scalar
        # queue is deep in silus and would stall batch 0's y otherwise
        qTs8 = cpool.tile([128, 2, NEP], F8, name="qTs8")
        for lc in range(2):
            pm = ps_mm.tile([128, 512], F32, name="pm_q3", tag="pm")
            for p in range(2):
                nc.tensor.matmul(pm[:],
                                 qw3_8[p][:, :, lc * 128:(lc + 1) * 128],
                                 qh2_8[:, 2 * p:2 * p + 2, :],
                                 start=(p == 0), stop=(p == 1), perf_mode=DR)
            nc.vector.tensor_scalar(qTs8[:, lc, :], pm[:],
                                    RS, qb3[:, lc:lc + 1],
                                    op0=ALU.mult, op1=ALU.add)
        # qwT8 = W3_k^T q  [h, e]: lets scores contract kh2 directly, so
        # the per-batch k L3 stage (and its kT8 cast) disappear entirely
        qwT8 = cpool.tile([128, 4, NEP], F8, name="qwT8")
        for hc in range(4):
            pm = ps_mm.tile([128, 512], F32, name="pm_qw", tag="pm")
            nc.tensor.matmul(pm[:], kw3T_8[0][:, :, hc * 128:(hc + 1) * 128],
                             qTs8[:], start=True, stop=True, perf_mode=DR)
            nc.vector.tensor_scalar_mul(qwT8[:, hc, :], pm[:], RS)

        # ---- remaining value/out-path weights + consts (sync ring) ----
        vw3 = wchunks("v_w3", HID, L)
        vb3_row = cpool.tile([1, L], F32R, name="vb3_row")
        nc.sync.dma_start(
            vb3_row[:], wd["v_b3"].rearrange("(a n) -> a n", a=1))
        ow1 = wchunks("o_w1", L, HID, eng=nc.scalar)
        ob1 = bias_col("o_b1", HID)
        ow2 = wchunks("o_w2", HID, L)
        ob2_row = cpool.tile([1, L], F32R, name="ob2_row")
        nc.sync.dma_start(
            ob2_row[:], wd["o_b2"].rearrange("(a n) -> a n", a=1))

        ident = cpool.tile([128, 128], F32, name="ident")
        masks.make_identity(nc, ident[:])
        ident_r = cpool.tile([128, 128], F32R, name="ident_r")
        nc.vector.tensor_copy(ident_r[:], ident[:])
        zeros_r = cpool.tile([128, 24], F32R, name="zeros_r")
        nc.vector.tensor_scalar_mul(zeros_r[:], ident[:, :24], 0.0)
        ones_blk = cpool.tile([128, 128], F32, name="ones_blk")
        nc.sync.dma_start(ones_blk[:], ones_d[:])
        ones_row = cpool.tile([1, 128], F32R, name="ones_row")
        nc.vector.tensor_copy(ones_row[:], ones_blk[:1, :])

        # bias broadcast tiles [128, 256] (one rank-1 matmul each)
        vb3_bc = cpool.tile([128, L], F32, name="vb3_bc")
        ob2_bc = cpool.tile([128, L], F32, name="ob2_bc")
        pbc = ps_u.tile([128, LA], F32, name="pbc", tag="pu")
        nc.tensor.matmul(pbc[:, :L], ones_row[:, :128], vb3_row[:],
                         start=True, stop=True)
        nc.vector.tensor_copy(vb3_bc[:], pbc[:, :L])
        pbc2 = ps_u.tile([128, LA], F32, name="pbc2", tag="pu")
        nc.tensor.matmul(pbc2[:, :L], ones_row[:, :128], ob2_row[:],
                         start=True, stop=True)
        nc.vector.tensor_copy(ob2_bc[:], pbc2[:, :L])

        # persistent per-batch tiles; zero the never-written pad rows of
        # the last token chunk once (y=0 there kills pad tokens in U)
        y = apool.tile([128, tch, NEP], F32R, name="y")
        v_aug = apool.tile([128, tch, LA], F32R, name="v_aug")
        last_off, last_sz = t_chunks[-1]
        if last_sz < 128:
            zsrc = cpool.tile([128, NEP], F32, name="zsrc")
            nc.gpsimd.memset(zsrc[:], 0.0)
            nc.vector.tensor_copy(y[:, tch - 1, :], zsrc[:, :])
            nc.vector.tensor_copy(v_aug[:, tch - 1, :], zsrc[:, :LA])
        nc.vector.tensor_copy(
            v_aug[:, :, L:LA],
            ones_blk[:, :tch * (LA - L)].rearrange(
                "p (a b) -> p a b", a=tch))

        def b_tail_fine(b, st):
            # final batch: no A-stream left to interleave, so pipeline the
            # B phases at chunk granularity (per-chunk oa/oh tiles avoid
            # whole-tile barriers between U -> norm -> transpose -> o MLP)
            oaT = dpool.tile([128, 2, NEP], F32R, name="oaT")
            nc.vector.tensor_copy(
                oaT[:, :, NE:NEP],
                zeros_r[:].rearrange("p (a b) -> p a b", a=2))
            for ec, (off, sz) in enumerate(E_CHUNKS):
                pu = ps_u.tile([128, LA], F32, name="pu_a", tag="pu")
                for nch in range(tch):
                    nc.tensor.matmul(pu[:sz, :], y[:, nch, off:off + sz],
                                     v_aug[:, nch, :],
                                     start=(nch == 0), stop=(nch == tch - 1))
                recip = dpool.tile([128, 1], F32, name="recip")
                nc.vector.reciprocal(recip[:sz], pu[:sz, L:L + 1])
                oa_c = dpool.tile([128, L], F32R, name=f"oa_c{ec}")
                nc.vector.tensor_scalar_mul(oa_c[:sz, :], pu[:sz, :L],
                                            recip[:sz])
                if ec % 2 == 0:
                    pt = ps_tp.tile([128, 2, 128], F32R, name="pt_a",
                                    tag="pt")
                else:
                    pt = ps_s.tile([128, 2, 128], F32R, name="pt_b",
                                   tag="ps")
                for lc in range(2):
                    nc.tensor.transpose(
                        pt[:, lc, :sz], oa_c[:sz, lc * 128:(lc + 1) * 128],
                        ident_r[:sz, :sz])
                nc.vector.tensor_copy(oaT[:, :, off:off + sz],
                                      pt[:, :, :sz])
            oh_c = []
            for oc in range(4):
                pm = ps_s.tile([128, 512], F32, name="pm_o1", tag="ps")
                for lc in range(2):
                    nc.tensor.matmul(pm[:],
                                     ow1[lc][:, oc * 128:(oc + 1) * 128],
                                     oaT[:, lc, :],
                                     start=(lc == 0), stop=(lc == 1))
                t = hpool.tile([128, NEP], F32R, name=f"oh_c{oc}")
                nc.scalar.activation(t[:], pm[:], AF.Silu,
                                     bias=ob1[:, oc:oc + 1])
                oh_c.append(t)
            yout = dpool.tile([128, 4, L], F32, name="yout")
            for pair in ((0, 1), (2, 3)):
                pus = [ps_u.tile([128, LA], F32, name="pu_o", tag="pu")
                       for _ in pair]
                for hc in range(4):
                    for j, ec in enumerate(pair):
                        off, sz = E_CHUNKS[ec]
                        nc.tensor.matmul(pus[j][:sz, :L],
                                         oh_c[hc][:, off:off + sz],
                                         ow2[hc][:], start=(hc == 0),
                                         stop=(hc == 3))
                for j, ec in enumerate(pair):
                    off, sz = E_CHUNKS[ec]
                    nc.vector.tensor_tensor(yout[:sz, ec, :],
                                            pus[j][:sz, :L],
                                            ob2_bc[:sz, :], op=ALU.add)
                    nc.sync.dma_start(out_d[b, off:off + sz],
                                      yout[:sz, ec, :])

        # ---- skewed main loop: B(b) interleaved with A(b+1); the next
        # batch's scores + v L3 are hoisted to the iteration tail so no
        # iteration starts on a serial scores->y->U or vh2->v_aug chain,
        # and the final (A-less) iteration runs a chunk-fused pipeline ----
        b1_scores(0, st_cur)
        b2_vl3(st_cur)
        for b in range(BL):
            st, st_nxt = st_cur, {}
            if b + 1 >= BL:
                b_tail_fine(b, st)
                break
            st_nxt["fldT"], st_nxt["fldT8"] = load_fld(b + 1)
            b3_u(st)
            a1_kl1(st_nxt)
            b4_oat(st)
            a2_vl1(st_nxt)
            b5_ol1(st)
            a3_kl2(st_nxt)
            a4_vl2(st_nxt)
            b6_ol2(b, st)
            b1_scores(b + 1, st_nxt)
            b2_vl3(st_nxt)
            st_cur = st_nxt

    split_excess_waits(nc)
    return nc


_NC_CACHE = {}


def _get_nc(cap=CAP0):
    if cap not in _NC_CACHE:
        _NC_CACHE[cap] = _build_nc(cap)
    return _NC_CACHE[cap]


def _prep(inputs):
    """Host-side shard prep: mask compaction + layout transposes only."""
    field = np.ascontiguousarray(inputs["field_atom_lat"], dtype=np.float32)
    mask = np.asarray(inputs["mask"]).astype(bool)
    cnts = mask.sum(axis=1)
    cmax = cnts.max()
    cap = CAP0 if cmax <= CAP0 else (CAP1 if cmax <= CAP1 else N)
    tch = (cap + 127) // 128

    idx = np.zeros((B, cap), np.int64)
    for b in range(B):
        nz = np.flatnonzero(mask[b])
        idx[b, :nz.size] = nz
    gathered = field[np.arange(B)[:, None], idx]          # [B, cap, FD]
    fldT = np.ascontiguousarray(
        gathered.transpose(0, 2, 1)).reshape(B, 2, 128, cap)

    padm = np.zeros((B, tch * 128), np.float32)
    padm[:, :cap] = (np.arange(cap)[None, :] < cnts[:, None])
    padm_cols = np.ascontiguousarray(
        padm.reshape(B, tch, 128).transpose(2, 1, 0))     # [128, tch, B]

    eT = np.zeros((ED, NEP), np.float32)
    eT[:, :NE] = np.ascontiguousarray(
        inputs["e_feat"], dtype=np.float32).T
    return cap, fldT, padm_cols, eT


def _kw3T(inputs):
    return np.ascontiguousarray(
        np.asarray(inputs["k_w3"], dtype=np.float32).T)


def _make_in_maps(inputs, fldT, padm_cols, eT):
    in_maps = []
    for c in range(NCORES):
        m = {
            "fldT": fldT[c * BL:(c + 1) * BL],
            "padm": np.ascontiguousarray(
                padm_cols[:, :, c * BL:(c + 1) * BL]),
            "e_featT": eT,
            "k_w3T": _kw3T(inputs),
        }
        for nm, _ in W_SPECS:
            m[nm] = np.ascontiguousarray(inputs[nm], dtype=np.float32)
        m["ones_in"] = np.ones((128, 128), dtype=np.float32)
        in_maps.append(m)
    return in_maps


def kernel(**inputs):
    cap, fldT, padm_cols, eT = _prep(inputs)
    nc = _get_nc(cap)
    in_maps = _make_in_maps(inputs, fldT, padm_cols, eT)
    res = run_bass_kernel_spmd(nc, in_maps, list(range(NCORES)))
    out = np.concatenate([res.results[c]["out"] for c in range(NCORES)],
                         axis=0)
    return out.astype(np.float32)
